# revision 1
# baseline (speedup 1.0000x reference)
"""Trainium2 Bass kernel for nn_CLS_5669356833410 (Wiener-deconv classifier).

Sharding: 8 cores = 4 samples x 2 halves. Core cid handles sample b=cid//2
and channel half h=cid%2 (channels 8h..8h+8 of the 16 reduced channels).
Core h=1 works in a vertically flipped world (host flips its inputs, host
unflips its output) so one SPMD program serves all cores; the g-chain is
split spatially across the pair and joined by a [16,9] AllReduce of pooled
partial sums. Each core emits a rank-8 partial of the final expand conv;
the host sums the two partials per sample (the unshard step).

FFTs are dense matmul DFTs with the data as the stationary operand
(A^T = X^T F form), rfft half-spectrum on the row axis, float32r matmuls.
"""

import dataclasses
import json as _json
import os

import numpy as np

B, NF, C, H, W, KS = 4, 64, 16, 256, 256, 21
HP = H + 2 * KS            # 298
NU = HP // 2 + 1           # 150
CH = 8
N_CORES = 8

PT = [(0, 128), (128, 128), (256, 42)]     # 298 partition tiling
UT = [(0, 128), (128, 22)]                 # 150 partition tiling

_CACHE = {}
LAST_RESULTS = None


# ---------------------------------------------------------------- patches
def _install_patches(bass, mybir, tile):
    if getattr(bass.Bass, "_nn_cls_patched", False):
        return
    from concourse.vector_clock import ScopedClock

    def _drain_and_barrier(self, tick_clock, wait_clock):
        nc = self.nc
        probe = nc.sync.nop(nofuse=True)
        wait_clock.add_sem_waits(
            probe.ins, ScopedClock({None: tick_clock.global_clock}))
        si = probe.ins.sync_info
        waits = list(si.on_wait) if si is not None else []
        if si is not None:
            si.on_wait.clear()
        for w in waits:
            n = nc.sync.nop(nofuse=True)
            if n.ins.sync_info is None:
                n.ins.sync_info = mybir.SyncInfo(on_wait=[w], on_update=[])
            else:
                n.ins.sync_info.on_wait.append(w)
        nc.sync.drain()
        nc.all_engine_barrier()
        assert self.sems is not None
        popped = nc._tile_sem_poison_stack.pop()
        assert popped is self._sem_poison
        nc.clear_and_free_semaphores(list(self.sems.allocated().values()))
        nc.all_engine_barrier()

    tile.TileContext._drain_and_barrier = _drain_and_barrier

    _orig = bass.Bass.to_json_bytes

    def _to_json_split(self, *a, **k):
        bir = _json.loads(_orig(self, *a, **k))
        cnt = 0
        for f in bir["functions"]:
            for blk in f["blocks"]:
                out = []
                for inst in blk["instructions"]:
                    si = inst.get("sync_info")
                    waits = si.get("on_wait") if si else None
                    cap = 0 if inst.get("opcode") == "Matmult" else 1
                    if waits and len(waits) > cap:
                        n = len(waits) - cap
                        extra, si["on_wait"] = waits[:n], waits[n:]
                        for w in extra:
                            cnt += 1
                            out.append({
                                "debug": inst.get("debug", 0),
                                "engine": inst["engine"], "ins": [],
                                "name": f"WS{cnt}", "opcode": "NoOp",
                                "outs": [],
                                "sync_info": {"on_update": [], "on_wait": [w]},
                            })
                    out.append(inst)
                blk["instructions"] = out
        return _json.dumps(bir).encode()

    bass.Bass.to_json_bytes = _to_json_split
    bass.Bass._nn_cls_patched = True


def _apv(ap, offset, dims):
    """Custom flat-element AP view: dims = [[step, count], ...]."""
    return dataclasses.replace(
        ap, offset=offset, ap=type(ap.ap)([list(d) for d in dims]))


# ---------------------------------------------------------------- consts
def _host_consts():
    N = HP
    i = np.arange(N, dtype=np.float64)
    u = np.arange(NU, dtype=np.float64)
    tw = 2.0 * np.pi / N
    c = {}
    a_iu = tw * np.outer(i, u)
    c["FH"] = np.concatenate([np.cos(a_iu), -np.sin(a_iu)], axis=1)
    a_jv = tw * np.outer(i, i)
    c["FC"] = np.cos(a_jv)
    c["FS"] = np.sin(a_jv)
    c["FSneg"] = -np.sin(a_jv)
    wu = np.full(NU, 2.0)
    wu[0] = wu[-1] = 1.0
    a_un = tw * np.outer(u, i)
    c["GHC"] = wu[:, None] * np.cos(a_un)
    c["GHS"] = wu[:, None] * np.sin(a_un)
    c["GHSneg"] = -c["GHS"]
    c["C2"] = np.cos(a_jv) / (N * N)
    c["S2neg"] = -np.sin(a_jv) / (N * N)
    s3 = np.arange(3.0) - 1.0
    c["E3r"] = np.cos(tw * np.outer(s3, i))
    c["E3i"] = -np.sin(tw * np.outer(s3, i))
    c["E3ip"] = np.sin(tw * np.outer(s3, i))
    c["Eu3c"] = np.cos(tw * np.outer(s3, u))
    c["Eu3s"] = -np.sin(tw * np.outer(s3, u))
    c["Eu3sneg"] = np.sin(tw * np.outer(s3, u))
    s21 = np.arange(float(KS)) - 10.0
    c["E21r"] = np.cos(tw * np.outer(s21, i))
    c["E21i"] = -np.sin(tw * np.outer(s21, i))
    c["Eu21c"] = np.cos(tw * np.outer(s21, u))
    c["Eu21s"] = -np.sin(tw * np.outer(s21, u))
    c["Eu21sneg"] = np.sin(tw * np.outer(s21, u))
    return {k: np.ascontiguousarray(v, np.float32) for k, v in c.items()}


def _row_weights(h):
    # g3 is 250x250; bins start 0/83/166, size 84, mean over 84*84.
    wrb = np.zeros((3, 125), np.float32)
    for yl in range(125):
        yt = yl if h == 0 else 249 - yl
        for ib in range(3):
            if 83 * ib <= yt < 83 * ib + 84:
                wrb[ib, yl] = 1.0 / (84.0 * 84.0)
    # expand with inner j-repeat: [16, 3, 125*3]
    wx = np.repeat(wrb[:, :, None], 3, axis=2).reshape(3, 375)
    return np.broadcast_to(wx[None], (16, 3, 375)).copy()


# ---------------------------------------------------------------- program
def _build_program(debug=False):
    import concourse.bass as bass
    import concourse.mybir as mybir
    from concourse import tile

    _install_patches(bass, mybir, tile)
    F32 = mybir.dt.float32
    F32R = mybir.dt.float32r
    AF = mybir.ActivationFunctionType
    ALU = mybir.AluOpType

    nc = bass.Bass("TRN2", target_bir_lowering=False, debug=False,
                   num_devices=N_CORES)
    din = {}

    def dinp(name, shape, dt=F32R):
        din[name] = nc.dram_tensor(name, list(shape), dt,
                                   kind="ExternalInput")
        return din[name]

    x64 = dinp("x64", [NF, H, W])
    dinp("kerT", [KS, KS], F32)
    dinp("wred24", [NF, 24])
    for nm in ("wg1", "wg2", "wg3"):
        dinp(nm, [96, 32])
    dinp("wg4p", [C, CH])
    dinp("wexp8", [128, NF])
    dinp("bred24", [24, 1], F32)
    for nm in ("bg1", "bg2", "bg3"):
        dinp(nm, [C, 1], F32)
    dinp("bg4p", [CH, 1], F32)
    dinp("bexp", [NF, 1], F32)
    dinp("selm", [C, 2], F32)
    dinp("wrbx", [C, 3, 375], F32)
    for nm, shp in [("FH", (HP, 300)), ("FC", (HP, HP)), ("FS", (HP, HP)),
                    ("FSneg", (HP, HP)), ("GHC", (NU, HP)), ("GHS", (NU, HP)),
                    ("GHSneg", (NU, HP)), ("C2", (HP, HP)),
                    ("S2neg", (HP, HP)), ("E3r", (3, HP)), ("E3i", (3, HP)), ("E3ip", (3, HP)),
                    ("Eu3c", (3, NU)), ("Eu3s", (3, NU)),
                    ("Eu3sneg", (3, NU))]:
        dinp(nm, shp)
    for nm, shp in [("E21r", (KS, HP)), ("E21i", (KS, HP)),
                    ("Eu21c", (KS, NU)), ("Eu21s", (KS, NU)),
                    ("Eu21sneg", (KS, NU))]:
        dinp(nm, shp, F32)

    BF16 = mybir.dt.bfloat16
    out_part = nc.dram_tensor("out_part", [NF, H, W], BF16,
                              kind="ExternalOutput")
    ccin = nc.dram_tensor("ccin", [C, 9], F32)
    ccout = nc.dram_tensor("ccout", [C, 9], F32)
    padrows = nc.dram_tensor("padrows", [2 * CH, W], F32R)
    dbg = {}
    if debug:
        for nm, shp in [("d_clsF", [128, 16 * W]), ("d_clsg", [128, 17 * W]),
                        ("d_g1", [128, 17 * W]), ("d_g2", [128, 17 * W]),
                        ("d_kp8", [CH, 9]), ("d_Kr", [NU, HP]),
                        ("d_Ki", [NU, HP]), ("d_Zr", [NU, HP]),
                        ("d_Zi", [NU, HP]), ("d_Br", [NU, HP]),
                        ("d_Bi", [NU, HP]), ("d_clear", [128, 16 * W]),
                        ("d_pool", [C, 9])]:
            dbg[nm] = nc.dram_tensor(nm, shp, F32, kind="ExternalOutput")

    with tile.TileContext(nc) as tc:
        with tc.tile_pool(name="persist", bufs=1) as pp:
            # ---------- constants to SBUF ----------
            def ctiles(name, cols, tiling):
                ts = []
                for (r0, rn) in tiling:
                    t = pp.tile([rn, cols], F32R, tag=f"{name}_{r0}", name=f"{name}_{r0}")
                    nc.sync.dma_start(t[:, :], din[name][r0:r0 + rn, :])
                    ts.append(t)
                return ts

            FHt = ctiles("FH", 300, PT)
            FCt = ctiles("FC", HP, PT)
            FSt = ctiles("FS", HP, PT)
            FSnt = ctiles("FSneg", HP, PT)
            GHCt = ctiles("GHC", HP, UT)
            GHSt = ctiles("GHS", HP, UT)
            GHSnt = ctiles("GHSneg", HP, UT)
            C2t = ctiles("C2", HP, PT)
            S2nt = ctiles("S2neg", HP, PT)

            def cload(name, shape, dt=F32R):
                t = pp.tile(list(shape), dt, tag=name, name=name)
                nc.sync.dma_start(t[:], din[name][:])
                return t

            E3r = cload("E3r", (3, HP))
            E3i = cload("E3i", (3, HP))
            E3ip = cload("E3ip", (3, HP))
            Eu3c = cload("Eu3c", (3, NU))
            Eu3s = cload("Eu3s", (3, NU))
            Eu3sn = cload("Eu3sneg", (3, NU))
            E21r = cload("E21r", (KS, HP), F32)
            E21i = cload("E21i", (KS, HP), F32)
            Eu21c = cload("Eu21c", (KS, NU), F32)
            Eu21s = cload("Eu21s", (KS, NU), F32)
            Eu21sn = cload("Eu21sneg", (KS, NU), F32)
            kerTs = cload("kerT", (KS, KS), F32)
            wred_s = cload("wred24", (NF, 24))
            wg_s = {k: cload(k, (96, 32)) for k in ("wg1", "wg2", "wg3")}
            wg4_s = cload("wg4p", (C, CH))
            wexp_s = cload("wexp8", (128, NF))
            bred_s = cload("bred24", (24, 1), F32)
            bg_s = {k: cload(k, (C, 1), F32) for k in ("bg1", "bg2", "bg3")}
            bg4_s = cload("bg4p", (CH, 1), F32)
            bexp_s = cload("bexp", (NF, 1), F32)
            selm_s = cload("selm", (C, 2), F32)
            wrbx_s = cload("wrbx", (C, 3, 375), F32)

            # ---------- persistent activations ----------
            cls_g = pp.tile([128, 17 * W], F32R, tag="cls_g")
            clsF = pp.tile([128, 16 * W], F32R, tag="clsF")
            g1p = pp.tile([128, 17 * W], F32R, tag="g1p")
            g2p = pp.tile([128, 17 * W], F32R, tag="g2p")
            clear = pp.tile([128, 16 * W], F32R, tag="clear")
            Krt = [pp.tile([rn, HP], F32, tag=f"Kr{r0}", name=f"Kr{r0}") for r0, rn in UT]
            Kit = [pp.tile([rn, HP], F32, tag=f"Ki{r0}", name=f"Ki{r0}") for r0, rn in UT]
            KD2t = [pp.tile([rn, HP], F32, tag=f"KD2{r0}", name=f"KD2{r0}") for r0, rn in UT]
            kp8 = pp.tile([CH, 9], F32R, tag="kp8")
            Tt = pp.tile([C, 125, 3], F32, tag="Tt")

            # ============ stage A: reduce conv ============
            with tc.tile_pool(name="sA", bufs=3) as pa, \
                 tc.tile_pool(name="psA", bufs=4, space="PSUM") as ppa:
                for kb in range(16):
                    xs = pa.tile([NF, 16 * W], F32R, tag="xs")
                    nc.sync.dma_start(xs[:, :],
                                      x64[:, 16 * kb:16 * kb + 16, :])
                    for ch in range(4):
                        ps = ppa.tile([24, 4 * W], F32, tag="psred")
                        for hf in range(2):
                            nc.tensor.matmul(
                                ps[:, hf * 512:(hf + 1) * 512], wred_s[:, :],
                                xs[:, ch * 1024 + hf * 512:
                                   ch * 1024 + (hf + 1) * 512],
                                start=True, stop=True)
                        bt = pa.tile([24, 4 * W], F32R, tag="bounce")
                        if ch % 2 == 0:
                            nc.scalar.activation(bt[:, :], ps[:, :],
                                                 AF.Identity,
                                                 bias=bred_s[:, 0:1])
                        else:
                            nc.vector.tensor_scalar_add(bt[:, :], ps[:, :],
                                                        bred_s[:, 0:1])
                        y0 = 16 * kb + 4 * ch
                        nc.sync.dma_start(
                            clsF[8 * kb:8 * kb + 8,
                                 4 * ch * W:(4 * ch + 4) * W],
                            bt[16:24, :])
                        for r in range(4):
                            y = y0 + r
                            if y >= 136:
                                continue
                            blk, off = divmod(y, 17)
                            nc.sync.dma_start(
                                cls_g[16 * blk:16 * blk + 16,
                                      off * W:(off + 1) * W],
                                bt[0:16, r * W:(r + 1) * W])

            # ============ Kf (per sample) ============
            with tc.tile_pool(name="skf", bufs=1) as pk, \
                 tc.tile_pool(name="pskf", bufs=2, space="PSUM") as ppk:
                psG = ppk.tile([KS, 1024], F32, tag="psG21")
                nc.tensor.matmul(psG[:, 0:HP], kerTs[:, :], E21r[:, :],
                                 start=True, stop=True)
                nc.tensor.matmul(psG[:, 512:512 + HP], kerTs[:, :],
                                 E21i[:, :], start=True, stop=True)
                G21 = pk.tile([KS, 2 * HP], F32, tag="G21")
                nc.vector.tensor_copy(G21[:, 0:HP], psG[:, 0:HP])
                nc.vector.tensor_copy(G21[:, HP:2 * HP],
                                      psG[:, 512:512 + HP])
                for it, (u0, un) in enumerate(UT):
                    psr = ppk.tile([un, HP], F32, tag="psKr")
                    psi = ppk.tile([un, HP], F32, tag="psKi")
                    nc.tensor.matmul(psr[:, :], Eu21c[:, u0:u0 + un],
                                     G21[:, 0:HP], start=True, stop=False)
                    nc.tensor.matmul(psr[:, :], Eu21sn[:, u0:u0 + un],
                                     G21[:, HP:2 * HP], start=False,
                                     stop=True)
                    nc.tensor.matmul(psi[:, :], Eu21c[:, u0:u0 + un],
                                     G21[:, HP:2 * HP], start=True,
                                     stop=False)
                    nc.tensor.matmul(psi[:, :], Eu21s[:, u0:u0 + un],
                                     G21[:, 0:HP], start=False, stop=True)
                    nc.vector.tensor_copy(Krt[it][:, :], psr[:, :])
                    nc.vector.tensor_copy(Kit[it][:, :], psi[:, :])
                    t1 = pk.tile([128, HP], F32, tag="kd_t1")
                    nc.scalar.activation(t1[0:un, :], psr[:, :], AF.Square)
                    nc.scalar.activation(KD2t[it][:, :], psi[:, :], AF.Square)
                    nc.vector.tensor_add(KD2t[it][:, :], KD2t[it][:, :],
                                         t1[0:un, :])

            # ============ g-chain (local frame, split across pair) ========
            def conv_layer(li, src, w_s, b_s, in_cols, out_rows, dst):
                out_cols = in_cols - 2
                with tc.tile_pool(name=f"g{li}", bufs=3) as pg, \
                     tc.tile_pool(name=f"psg{li}", bufs=4,
                                  space="PSUM") as ppg:
                    nblk = (out_rows + 16) // 17
                    for bk in range(nblk):
                        r0 = 17 * bk
                        rows = min(17, out_rows - r0)
                        r96 = pg.tile([96, 19 * W], F32R, tag=f"r96_{li}")
                        for dyy in range(2):
                            need = rows + 2 if dyy == 0 else rows
                            got = 0
                            while got < need:
                                y = r0 + dyy + got
                                sb, so = divmod(y, 17)
                                n = min(17 - so, need - got)
                                for dx in range(3):
                                    nc.sync.dma_start(
                                        r96[48 * dyy + 16 * dx:
                                            48 * dyy + 16 * dx + 16,
                                            got * W:(got + n) * W - dx],
                                        src[16 * sb:16 * sb + 16,
                                            so * W + dx:(so + n) * W])
                                got += n
                        for c0 in range(0, rows, 2):
                            rr = min(2, rows - c0)
                            ps = ppg.tile([16, 2, out_cols], F32,
                                          tag=f"ps_{li}")
                            rhs3 = r96[:, c0 * W:(c0 + rr) * W].rearrange(
                                "p (r x) -> p r x", r=rr)
                            nc.tensor.matmul(
                                ps[:, 0:rr, :], w_s[:, 0:16],
                                rhs3[:, :, 0:out_cols],
                                start=True, stop=False)
                            rhs2 = r96[0:48, (c0 + 2) * W:
                                       (c0 + 2 + rr) * W].rearrange(
                                "p (r x) -> p r x", r=rr)
                            nc.tensor.matmul(
                                ps[:, 0:rr, :], w_s[0:48, 16:32],
                                rhs2[:, :, 0:out_cols],
                                start=False, stop=True)
                            if dst is not None:
                                bt = pg.tile([16, 2, out_cols], F32R,
                                             tag=f"bt_{li}")
                                nc.scalar.activation(
                                    bt[:, 0:rr, :], ps[:, 0:rr, :],
                                    AF.Lrelu, bias=b_s[:, 0:1], alpha=0.1)
                                for r in range(rr):
                                    yo = r0 + c0 + r
                                    db_, do_ = divmod(yo, 17)
                                    nc.sync.dma_start(
                                        dst[16 * db_:16 * db_ + 16,
                                            do_ * W:do_ * W + out_cols],
                                        bt[:, r, :])
                            else:
                                # g3: overlapping column-bin sums from PSUM
                                for r in range(rr):
                                    yo = r0 + c0 + r
                                    full = ps[:, :, :]
                                    binv = _apv(full, r * out_cols,
                                                [list(full.ap[0]),
                                                 [83, 3], [1, 84]])
                                    nc.vector.tensor_reduce(
                                        Tt[:, yo, :], binv,
                                        mybir.AxisListType.X, ALU.add)

            conv_layer(1, cls_g, wg_s["wg1"], bg_s["bg1"], 256, 129, g1p)
            conv_layer(2, g1p, wg_s["wg2"], bg_s["bg2"], 254, 127, g2p)
            conv_layer(3, g2p, wg_s["wg3"], bg_s["bg3"], 252, 125, None)

            # ---- pool partials, AllReduce, kernel_P ----
            with tc.tile_pool(name="spool", bufs=1) as pq, \
                 tc.tile_pool(name="pspool", bufs=2, space="PSUM") as ppq:
                Sp = pq.tile([C, 3, 3], F32, tag="Spart")
                tmp = pq.tile([C, 125, 3], F32, tag="ptmp")
                for ib in range(3):
                    nc.vector.tensor_tensor(
                        tmp[:, :, :], Tt[:, :, :],
                        wrbx_s[:, ib, :].rearrange("p (y j) -> p y j", j=3),
                        ALU.mult)
                    tv = tmp[:, :, :]
                    swapped = _apv(tv, 0, [list(tv.ap[0]), [1, 3], [3, 125]])
                    nc.vector.tensor_reduce(Sp[:, ib, :], swapped,
                                            mybir.AxisListType.X, ALU.add)
                nc.sync.dma_start(ccin[:, :], Sp[:, :, :])
                nc.gpsimd.collective_compute(
                    "AllReduce", ALU.add,
                    replica_groups=[[0, 1], [2, 3], [4, 5], [6, 7]],
                    ins=[ccin[:, :]], outs=[ccout[:, :]])
                pooled = pq.tile([C, 9], F32, tag="pooled")
                nc.sync.dma_start(pooled[:, :], ccout[:, :])
                if debug:
                    nc.sync.dma_start(dbg["d_pool"][:, :], pooled[:, :])
                pflip = pq.tile([C, 9], F32, tag="pflip")
                for ib in range(3):
                    nc.vector.tensor_copy(
                        pflip[:, 3 * ib:3 * ib + 3],
                        pooled[:, 3 * (2 - ib):3 * (2 - ib) + 3])
                pmine = pq.tile([C, 16], F32R, tag="pmine")
                nc.vector.tensor_scalar_mul(pmine[:, 9:16], pooled[:, 0:7],
                                            0.0)
                psel = pq.tile([C, 9], F32, tag="psel")
                nc.vector.tensor_scalar_mul(psel[:, :], pooled[:, :],
                                            selm_s[:, 0:1])
                nc.vector.scalar_tensor_tensor(
                    psel[:, :], pflip[:, :], selm_s[:, 1:2], psel[:, :],
                    ALU.mult, ALU.add)
                # add b_g3 (pool commutes with the bias)
                nc.vector.tensor_scalar_add(pmine[:, 0:9], psel[:, :],
                                            bg_s["bg3"][:, 0:1])
                psk = ppq.tile([CH, 16], F32, tag="psk")
                nc.tensor.matmul(psk[:, :], wg4_s[:, :], pmine[:, :],
                                 start=True, stop=True)
                kpe = pq.tile([CH, 9], F32, tag="kpe")
                nc.scalar.activation(kpe[:, :], psk[:, 0:9], AF.Exp,
                                     bias=bg4_s[:, 0:1])
                nsum = pq.tile([CH, 1], F32, tag="nsum")
                nc.vector.tensor_reduce(nsum[:, :], kpe[:, :],
                                        mybir.AxisListType.X, ALU.add,
                                        negate=True)
                nmean = pq.tile([CH, 1], F32, tag="nmean")
                nc.scalar.mul(nmean[:, :], nsum[:, :], 1.0 / 9.0)
                nc.vector.tensor_scalar_add(kp8[:, :], kpe[:, :],
                                            nmean[:, 0:1])
                if debug:
                    nc.gpsimd.dma_start(dbg["d_kp8"][:, :], kp8[:, :])

            if debug:
                nc.gpsimd.dma_start(dbg["d_clsF"][:, :], clsF[:, :])
                nc.gpsimd.dma_start(dbg["d_clsg"][:, :], cls_g[:, :])
                nc.gpsimd.dma_start(dbg["d_g1"][:, :], g1p[:, :])
                nc.gpsimd.dma_start(dbg["d_g2"][:, :], g2p[:, :])
                for it, (u0, un) in enumerate(UT):
                    nc.sync.dma_start(dbg["d_Kr"][u0:u0 + un, :],
                                      Krt[it][:, :])
                    nc.sync.dma_start(dbg["d_Ki"][u0:u0 + un, :],
                                      Kit[it][:, :])

            # ============ FFT / Wiener per channel ============
            with tc.tile_pool(name="fft", bufs=2) as pf, \
                 tc.tile_pool(name="fftx", bufs=3) as pfx, \
                 tc.tile_pool(name="psf", bufs=2, space="PSUM") as ppf, \
                 tc.tile_pool(name="psf1", bufs=2, space="PSUM") as ppf1:
                for cix in range(CH):
                    # ---- build padded X ----
                    Xt = [pfx.tile([rn, HP], F32R, tag=f"X{r0}", name=f"X{r0}")
                          for r0, rn in PT]
                    for sb in range(16):
                        srow = clsF[8 * sb + cix:8 * sb + cix + 1, :]
                        sv = srow.rearrange("p (y x) -> p y x", x=W)
                        yd0 = 21 + 16 * sb
                        done = 0
                        while done < 16:
                            yd = yd0 + done
                            ti = 0 if yd < 128 else (1 if yd < 256 else 2)
                            t0 = PT[ti][0]
                            n = min(16 - done, t0 + PT[ti][1] - yd)
                            nc.sync.dma_start(
                                Xt[ti][yd - t0:yd - t0 + n, 21:21 + W],
                                sv[0:1, done:done + n, :])
                            done += n
                    nc.sync.dma_start(padrows[2 * cix:2 * cix + 1, :],
                                      clsF[cix:cix + 1, 0:W])
                    nc.sync.dma_start(
                        padrows[2 * cix + 1:2 * cix + 2, :],
                        clsF[8 * 15 + cix:8 * 15 + cix + 1, 15 * W:16 * W])
                    nc.sync.dma_start(
                        Xt[0][0:21, 21:21 + W],
                        padrows[2 * cix:2 * cix + 1, :]
                        .broadcast_to([21, W]))
                    nc.sync.dma_start(
                        Xt[2][21:42, 21:21 + W],
                        padrows[2 * cix + 1:2 * cix + 2, :]
                        .broadcast_to([21, W]))
                    for ti, (r0, rn) in enumerate(PT):
                        # col pads: out = in*0 + colvalue  (per-partition
                        # scalar broadcast along free dim)
                        nc.vector.tensor_scalar(
                            Xt[ti][:, 0:21], Xt[ti][:, 21:42], 0.0,
                            Xt[ti][:, 21:22].bitcast(F32), ALU.mult,
                            ALU.add)
                        nc.vector.tensor_scalar(
                            Xt[ti][:, 277:HP], Xt[ti][:, 255:276], 0.0,
                            Xt[ti][:, 276:277].bitcast(F32), ALU.mult,
                            ALU.add)

                    # ---- stage 1: A^T[j, u] ----
                    At = [pfx.tile([rn, 300], F32R, tag=f"At{r0}", name=f"At{r0}")
                          for r0, rn in PT]
                    for jt, (j0, jn) in enumerate(PT):
                        psA = ppf.tile([128, 300], F32, tag="psPr", name="psA")[0:jn]
                        for it in range(3):
                            nc.tensor.matmul(psA[:, :],
                                             Xt[it][:, j0:j0 + jn],
                                             FHt[it][:, :],
                                             start=(it == 0), stop=(it == 2))
                        nc.scalar.copy(At[jt][:, :], psA[:, :])

                    # ---- Pf (contract r first; P3 in natural layout) ----
                    P3 = pf.tile([3, 3], F32R, tag="P3")
                    nc.sync.dma_start(
                        P3[:, :],
                        kp8[cix:cix + 1, :].rearrange("p (r s) -> p r s",
                                                      s=3))
                    psGur = ppf1.tile([128, HP], F32, tag="psBr",
                                      name="psGur")[0:3]
                    psGui = ppf1.tile([128, HP], F32, tag="psBi",
                                      name="psGui")[0:3]
                    nc.tensor.matmul(psGur[:, 0:NU], P3[:, :], Eu3c[:, :],
                                     start=True, stop=True)
                    nc.tensor.matmul(psGui[:, 0:NU], P3[:, :], Eu3s[:, :],
                                     start=True, stop=True)
                    G3 = pf.tile([3, 2 * NU], F32R, tag="G3")
                    nc.vector.tensor_copy(G3[:, 0:NU], psGur[:, 0:NU])
                    nc.vector.tensor_copy(G3[:, NU:2 * NU], psGui[:, 0:NU])

                    # ---- stage 2 + Wiener per u-tile ----
                    Zr = [pf.tile([rn, HP], F32R, tag=f"Zr{r0}", name=f"Zr{r0}")
                          for r0, rn in UT]
                    Zi = [pf.tile([rn, HP], F32R, tag=f"Zi{r0}", name=f"Zi{r0}")
                          for r0, rn in UT]
                    for it, (u0, un) in enumerate(UT):
                        psPr = ppf.tile([128, HP], F32, tag="psPr", name="psPr")[0:un]
                        psPi = ppf.tile([128, HP], F32, tag="psPi", name="psPi")[0:un]
                        nc.tensor.matmul(psPr[:, :],
                                         G3[:, u0:u0 + un],
                                         E3r[:, :], start=True, stop=False)
                        nc.tensor.matmul(psPr[:, :],
                                         G3[:, NU + u0:NU + u0 + un],
                                         E3ip[:, :], start=False, stop=True)
                        nc.tensor.matmul(psPi[:, :],
                                         G3[:, u0:u0 + un],
                                         E3i[:, :], start=True, stop=False)
                        nc.tensor.matmul(psPi[:, :],
                                         G3[:, NU + u0:NU + u0 + un],
                                         E3r[:, :], start=False, stop=True)
                        psBr = ppf1.tile([128, HP], F32, tag="psBr", name="psBr")[0:un]
                        psBi = ppf1.tile([128, HP], F32, tag="psBi", name="psBi")[0:un]
                        for jt, (j0, jn) in enumerate(PT):
                            Ar = At[jt][:, u0:u0 + un]
                            Ai = At[jt][:, 150 + u0:150 + u0 + un]
                            nc.tensor.matmul(psBr[:, :], Ar, FCt[jt][:, :],
                                             start=(jt == 0), stop=False)
                            nc.tensor.matmul(psBr[:, :], Ai, FSt[jt][:, :],
                                             start=False, stop=(jt == 2))
                            nc.tensor.matmul(psBi[:, :], Ai, FCt[jt][:, :],
                                             start=(jt == 0), stop=False)
                            nc.tensor.matmul(psBi[:, :], Ar, FSnt[jt][:, :],
                                             start=False, stop=(jt == 2))
                        if debug and cix == 0:
                            tb = pf.tile([128, HP], F32, tag="tbdbg")
                            nc.vector.tensor_copy(tb[0:un, :], psBr[:, :])
                            nc.sync.dma_start(dbg["d_Br"][u0:u0 + un, :],
                                              tb[0:un, :])
                            tb2 = pf.tile([128, HP], F32, tag="tbdbg2")
                            nc.vector.tensor_copy(tb2[0:un, :], psBi[:, :])
                            nc.sync.dma_start(dbg["d_Bi"][u0:u0 + un, :],
                                              tb2[0:un, :])
                        sq1 = pf.tile([128, HP], F32, tag="sq1")
                        sq2 = pf.tile([128, HP], F32, tag="sq2")
                        nc.scalar.activation(sq1[0:un, :], psPr[:, :],
                                             AF.Square)
                        nc.scalar.activation(sq2[0:un, :], psPi[:, :],
                                             AF.Square)
                        nc.vector.tensor_add(sq1[0:un, :], sq1[0:un, :],
                                             sq2[0:un, :])
                        nc.vector.tensor_add(sq1[0:un, :], sq1[0:un, :],
                                             KD2t[it][:, :])
                        rec = pf.tile([128, HP], F32, tag="rec")
                        nc.vector.reciprocal(rec[0:un, :], sq1[0:un, :])
                        m1 = pf.tile([128, HP], F32, tag="m1")
                        m2 = pf.tile([128, HP], F32, tag="m2")
                        nc.vector.tensor_tensor(m1[0:un, :], psBr[:, :],
                                                Krt[it][:, :], ALU.mult)
                        nc.vector.tensor_tensor(m2[0:un, :], psBi[:, :],
                                                Kit[it][:, :], ALU.mult)
                        nc.vector.tensor_add(m1[0:un, :], m1[0:un, :],
                                             m2[0:un, :])
                        nc.vector.tensor_tensor(Zr[it][:, :], m1[0:un, :],
                                                rec[0:un, :], ALU.mult)
                        nc.vector.tensor_tensor(m1[0:un, :], psBi[:, :],
                                                Krt[it][:, :], ALU.mult)
                        nc.vector.tensor_tensor(m2[0:un, :], psBr[:, :],
                                                Kit[it][:, :], ALU.mult)
                        nc.vector.tensor_tensor(m1[0:un, :], m1[0:un, :],
                                                m2[0:un, :], ALU.subtract)
                        nc.vector.tensor_tensor(Zi[it][:, :], m1[0:un, :],
                                                rec[0:un, :], ALU.mult)
                    if debug and cix == 0:
                        for it, (u0, un) in enumerate(UT):
                            nc.gpsimd.dma_start(dbg["d_Zr"][u0:u0 + un, :],
                                                Zr[it][:, :])
                            nc.gpsimd.dma_start(dbg["d_Zi"][u0:u0 + un, :],
                                                Zi[it][:, :])

                    # ---- inverse stage 1: V^T[v, n] ----
                    Vr = [pf.tile([rn, HP], F32R, tag=f"Vr{r0}", name=f"Vr{r0}")
                          for r0, rn in PT]
                    Vi = [pf.tile([rn, HP], F32R, tag=f"Vi{r0}", name=f"Vi{r0}")
                          for r0, rn in PT]
                    for vt, (v0, vn) in enumerate(PT):
                        psVr = ppf.tile([128, HP], F32, tag="psPr", name="psVr")[0:vn]
                        psVi = ppf.tile([128, HP], F32, tag="psPi", name="psVi")[0:vn]
                        for it, (u0, un) in enumerate(UT):
                            zr = Zr[it][:, v0:v0 + vn]
                            zi = Zi[it][:, v0:v0 + vn]
                            nc.tensor.matmul(psVr[:, :], zr, GHCt[it][:, :],
                                             start=(it == 0), stop=False)
                            nc.tensor.matmul(psVr[:, :], zi, GHSnt[it][:, :],
                                             start=False, stop=(it == 1))
                            nc.tensor.matmul(psVi[:, :], zi, GHCt[it][:, :],
                                             start=(it == 0), stop=False)
                            nc.tensor.matmul(psVi[:, :], zr, GHSt[it][:, :],
                                             start=False, stop=(it == 1))
                        nc.scalar.copy(Vr[vt][:, :], psVr[:, :])
                        nc.vector.tensor_copy(Vi[vt][:, :], psVi[:, :])

                    # ---- inverse stage 2 + crop + remap ----
                    for nt in range(2):
                        n0 = 21 + 128 * nt
                        psD = ppf.tile([128, HP], F32, tag="psPr", name="psD")
                        for vt, (v0, vn) in enumerate(PT):
                            nc.tensor.matmul(psD[:, :],
                                             Vr[vt][:, n0:n0 + 128],
                                             C2t[vt][:, :],
                                             start=(vt == 0), stop=False)
                            nc.tensor.matmul(psD[:, :],
                                             Vi[vt][:, n0:n0 + 128],
                                             S2nt[vt][:, :],
                                             start=False, stop=(vt == 2))
                        deb = pf.tile([128, W], F32R, tag="deb")
                        nc.vector.tensor_copy(deb[:, :], psD[:, 21:277])
                        dv = clear[:, :]
                        dst = _apv(dv, (cix + 64 * nt) * (16 * W),
                                   [[8 * 16 * W, 8], [W, 16], [1, W]])
                        nc.sync.dma_start(dst, deb[:, :])

            if debug:
                nc.gpsimd.dma_start(dbg["d_clear"][:, :], clear[:, :])

            # ============ expand (rank-8 partial) ============
            with tc.tile_pool(name="sE", bufs=4) as pe, \
                 tc.tile_pool(name="psE", bufs=4, space="PSUM") as ppe:
                for yb in range(16):
                    stg = pe.tile([CH, 16 * W], F32R, tag="stg")
                    nc.sync.dma_start(stg[:, :],
                                      clear[8 * yb:8 * yb + 8, :])
                    for ck in range(8):
                        psE = ppe.tile([NF, 512], F32, tag="psE")
                        nc.tensor.matmul(
                            psE[:, :], wexp_s[0:8, :],
                            stg[:, 512 * ck:512 * (ck + 1)],
                            start=True, stop=True)
                        ob = pe.tile([NF, 512], BF16, tag="ob")
                        if ck % 2 == 0:
                            nc.vector.tensor_scalar_add(ob[:, :], psE[:, :],
                                                        bexp_s[:, 0:1])
                        else:
                            nc.scalar.activation(ob[:, :], psE[:, :],
                                                 AF.Identity,
                                                 bias=bexp_s[:, 0:1])
                        y = 16 * yb + 2 * ck
                        nc.sync.dma_start(out_part[:, y:y + 2, :],
                                          ob[:, :])

    return nc


# ---------------------------------------------------------------- host
def _core_inputs(inputs, cid, consts):
    b, h = cid // 2, cid % 2
    flip = (h == 1)
    x = inputs["x"][b]
    ker = inputs["kernel"][b, 0]
    if flip:
        x = np.flip(x, axis=1)
        ker = np.flip(ker, axis=0)
    wr = inputs["w_reduce"][:, :, 0, 0]          # [16, 64]
    wred24 = np.zeros((NF, 24), np.float32)
    wred24[:, 0:16] = wr.T
    wred24[:, 16:24] = wr[8 * h:8 * h + 8].T

    def packg(wg):                               # [o,c,3,3] -> [96, 32]
        w = np.flip(wg, axis=2) if flip else wg
        out = np.zeros((96, 32), np.float32)
        for dy in range(3):
            for dx in range(3):
                for cc in range(16):
                    if dy < 2:
                        out[48 * dy + 16 * dx + cc, 0:16] = w[:, cc, dy, dx]
                    else:
                        out[16 * dx + cc, 16:32] = w[:, cc, dy, dx]
        return out

    wexp = inputs["w_expand"][:, :, 0, 0]        # [64, 16]
    wexp8 = np.tile(wexp[:, 8 * h:8 * h + 8].T, (16, 1)).astype(np.float32)

    m = {
        "x64": np.ascontiguousarray(x, np.float32),
        "kerT": np.ascontiguousarray(ker.T, np.float32),
        "wred24": wred24,
        "wg1": packg(inputs["w_g1"]),
        "wg2": packg(inputs["w_g2"]),
        "wg3": packg(inputs["w_g3"]),
        "wg4p": np.ascontiguousarray(
            inputs["w_g4"][8 * h:8 * h + 8, :, 0, 0].T, np.float32),
        "wexp8": wexp8,
        "bred24": np.concatenate(
            [inputs["b_reduce"], inputs["b_reduce"][8 * h:8 * h + 8]]
        ).reshape(24, 1).astype(np.float32),
        "bg1": inputs["b_g1"].reshape(C, 1).astype(np.float32),
        "bg2": inputs["b_g2"].reshape(C, 1).astype(np.float32),
        "bg3": inputs["b_g3"].reshape(C, 1).astype(np.float32),
        "bg4p": inputs["b_g4"][8 * h:8 * h + 8].reshape(CH, 1)
                .astype(np.float32),
        "bexp": (inputs["b_expand"] if h == 0
                 else np.zeros(NF)).reshape(NF, 1).astype(np.float32),
        "selm": np.tile(np.array([[1.0, 0.0]] if h == 0 else [[0.0, 1.0]],
                                 np.float32), (C, 1)),
        "wrbx": _row_weights(h),
    }
    m.update(consts)
    return m


def kernel(**inputs):
    inputs = {k: np.asarray(v) for k, v in inputs.items()}
    key = "prog"
    if key not in _CACHE:
        _CACHE[key] = _build_program(debug=False)
        _CACHE["consts"] = _host_consts()
    nc = _CACHE[key]
    consts = _CACHE["consts"]

    from concourse.bass_utils import run_bass_kernel_spmd
    in_maps = [_core_inputs(inputs, cid, consts) for cid in range(N_CORES)]
    trace = os.environ.get("NN_CLS_TRACE", "0") == "1"
    kw = {}
    if trace:
        kw = dict(trace=True, trace_cores=list(range(N_CORES)),
                  stitch_traces=True)
    try:
        res = run_bass_kernel_spmd(nc, in_maps,
                                   core_ids=list(range(N_CORES)), **kw)
    except ModuleNotFoundError:
        res = run_bass_kernel_spmd(nc, in_maps,
                                   core_ids=list(range(N_CORES)))
    global LAST_RESULTS
    LAST_RESULTS = res

    out = np.zeros((B, NF, H, W), np.float32)
    for b in range(B):
        pa = np.asarray(res.results[2 * b]["out_part"], np.float32)
        pb = np.asarray(res.results[2 * b + 1]["out_part"], np.float32)
        out[b] = pa + np.flip(pb, axis=1)
    return out



# revision 7
# speedup vs baseline: 7981.8285x; 7981.8285x over previous
"""Trainium2 Bass kernel for nn_CLS_5669356833410 (Wiener-deconv classifier).

Sharding: 8 cores = 4 samples x 2 halves. Core cid handles sample b=cid//2
and channel half h=cid%2 (channels 8h..8h+8 of the 16 reduced channels).
Core h=1 works in a vertically flipped world (host flips its inputs, host
unflips its output) so one SPMD program serves all cores; the g-chain is
split spatially across the pair and joined by a [16,9] AllReduce of pooled
partial sums. Each core emits a rank-8 partial of the final expand conv;
the host sums the two partials per sample (the unshard step).

FFTs are dense matmul DFTs with the data as the stationary operand
(A^T = X^T F form), rfft half-spectrum on the row axis, float32r matmuls.

Runtime layer: the DFT basis matrices are baked into the NEFF via
inline_tensor (loaded to HBM once at model-load), the jitted executable is
compiled once and cached, and no donated zero output buffers are uploaded
(the NKI wrapper allocates outputs on-device). A chained-K executable is
available for measuring true NEFF execution time by slope.
"""

import dataclasses
import json as _json

import numpy as np

B, NF, C, H, W, KS = 4, 64, 16, 256, 256, 21
HP = H + 2 * KS            # 298
NU = HP // 2 + 1           # 150
CH = 8
N_CORES = 8

PT = [(0, 128), (128, 128), (256, 42)]     # 298 partition tiling
UT = [(0, 128), (128, 22)]                 # 150 partition tiling

_RT = {}


# ---------------------------------------------------------------- patches
def _install_patches(bass, mybir, tile):
    if getattr(bass.Bass, "_nn_cls_patched", False):
        return
    from concourse.vector_clock import ScopedClock

    def _drain_and_barrier(self, tick_clock, wait_clock):
        nc = self.nc
        probe = nc.sync.nop(nofuse=True)
        wait_clock.add_sem_waits(
            probe.ins, ScopedClock({None: tick_clock.global_clock}))
        si = probe.ins.sync_info
        waits = list(si.on_wait) if si is not None else []
        if si is not None:
            si.on_wait.clear()
        for w in waits:
            n = nc.sync.nop(nofuse=True)
            if n.ins.sync_info is None:
                n.ins.sync_info = mybir.SyncInfo(on_wait=[w], on_update=[])
            else:
                n.ins.sync_info.on_wait.append(w)
        nc.sync.drain()
        nc.all_engine_barrier()
        assert self.sems is not None
        popped = nc._tile_sem_poison_stack.pop()
        assert popped is self._sem_poison
        nc.clear_and_free_semaphores(list(self.sems.allocated().values()))
        nc.all_engine_barrier()

    tile.TileContext._drain_and_barrier = _drain_and_barrier

    _orig = bass.Bass.to_json_bytes

    def _to_json_split(self, *a, **k):
        bir = _json.loads(_orig(self, *a, **k))
        cnt = 0
        for f in bir["functions"]:
            for blk in f["blocks"]:
                out = []
                for inst in blk["instructions"]:
                    si = inst.get("sync_info")
                    waits = si.get("on_wait") if si else None
                    cap = 0 if inst.get("opcode") == "Matmult" else 1
                    if waits and len(waits) > cap:
                        n = len(waits) - cap
                        extra, si["on_wait"] = waits[:n], waits[n:]
                        for w in extra:
                            cnt += 1
                            out.append({
                                "debug": inst.get("debug", 0),
                                "engine": inst["engine"], "ins": [],
                                "name": f"WS{cnt}", "opcode": "NoOp",
                                "outs": [],
                                "sync_info": {"on_update": [], "on_wait": [w]},
                            })
                    out.append(inst)
                blk["instructions"] = out
        return _json.dumps(bir).encode()

    bass.Bass.to_json_bytes = _to_json_split
    bass.Bass._nn_cls_patched = True


def _apv(ap, offset, dims):
    """Custom flat-element AP view: dims = [[step, count], ...]."""
    return dataclasses.replace(
        ap, offset=offset, ap=type(ap.ap)([list(d) for d in dims]))


# ---------------------------------------------------------------- consts
def _host_consts():
    N = HP
    i = np.arange(N, dtype=np.float64)
    u = np.arange(NU, dtype=np.float64)
    tw = 2.0 * np.pi / N
    c = {}
    a_iu = tw * np.outer(i, u)
    c["FH"] = np.concatenate([np.cos(a_iu), -np.sin(a_iu)], axis=1)
    a_jv = tw * np.outer(i, i)
    c["FC"] = np.cos(a_jv)
    c["FS"] = np.sin(a_jv)
    c["FSneg"] = -np.sin(a_jv)
    wu = np.full(NU, 2.0)
    wu[0] = wu[-1] = 1.0
    a_un = tw * np.outer(u, i)
    c["GHC"] = wu[:, None] * np.cos(a_un)
    c["GHS"] = wu[:, None] * np.sin(a_un)
    c["GHSneg"] = -c["GHS"]
    c["C2"] = np.cos(a_jv) / (N * N)
    c["S2neg"] = -np.sin(a_jv) / (N * N)
    s3 = np.arange(3.0) - 1.0
    c["E3r"] = np.cos(tw * np.outer(s3, i))
    c["E3i"] = -np.sin(tw * np.outer(s3, i))
    c["E3ip"] = np.sin(tw * np.outer(s3, i))
    c["Eu3c"] = np.cos(tw * np.outer(s3, u))
    c["Eu3s"] = -np.sin(tw * np.outer(s3, u))
    c["Eu3sneg"] = np.sin(tw * np.outer(s3, u))
    s21 = np.arange(float(KS)) - 10.0
    c["E21r"] = np.cos(tw * np.outer(s21, i))
    c["E21i"] = -np.sin(tw * np.outer(s21, i))
    c["Eu21c"] = np.cos(tw * np.outer(s21, u))
    c["Eu21s"] = -np.sin(tw * np.outer(s21, u))
    c["Eu21sneg"] = np.sin(tw * np.outer(s21, u))
    return {k: np.ascontiguousarray(v, np.float32) for k, v in c.items()}


def _row_weights(h):
    # g3 is 250x250; bins start 0/83/166, size 84, mean over 84*84.
    wrb = np.zeros((3, 125), np.float32)
    for yl in range(125):
        yt = yl if h == 0 else 249 - yl
        for ib in range(3):
            if 83 * ib <= yt < 83 * ib + 84:
                wrb[ib, yl] = 1.0 / (84.0 * 84.0)
    # expand with inner j-repeat: [16, 3, 125*3]
    wx = np.repeat(wrb[:, :, None], 3, axis=2).reshape(3, 375)
    return np.broadcast_to(wx[None], (16, 3, 375)).copy()


# ---------------------------------------------------------------- program
def _build_program():
    import concourse.bass as bass
    import concourse.mybir as mybir
    from concourse import tile

    _install_patches(bass, mybir, tile)
    F32 = mybir.dt.float32
    F32R = mybir.dt.float32r
    AF = mybir.ActivationFunctionType
    ALU = mybir.AluOpType

    consts = _host_consts()

    nc = bass.Bass("TRN2", target_bir_lowering=False, debug=False,
                   num_devices=N_CORES)
    din = {}

    def dinp(name, shape, dt=F32R):
        din[name] = nc.dram_tensor(name, list(shape), dt,
                                   kind="ExternalInput")
        return din[name]

    x64 = dinp("x64", [NF, H, W])
    dinp("kerT", [KS, KS], F32)
    dinp("wred24", [NF, 24])
    for nm in ("wg1", "wg2", "wg3"):
        dinp(nm, [96, 32])
    dinp("wg4p", [C, CH])
    dinp("wexp8", [128, NF])
    dinp("bred24", [24, 1], F32)
    for nm in ("bg1", "bg2", "bg3"):
        dinp(nm, [C, 1], F32)
    dinp("bg4p", [CH, 1], F32)
    dinp("bexp", [NF, 1], F32)
    dinp("selm", [C, 2], F32)
    dinp("wrbx", [C, 3, 375], F32)
    for nm in ("FH", "FC", "FS", "FSneg", "GHC", "GHS", "GHSneg",
               "C2", "S2neg", "E3r", "E3i", "E3ip", "Eu3c", "Eu3s",
               "Eu3sneg", "E21r", "E21i", "Eu21c", "Eu21s", "Eu21sneg"):
        din[nm] = nc.inline_tensor(consts[nm], name=nm)

    BF16 = mybir.dt.bfloat16
    out_part = nc.dram_tensor("out_part", [NF, H, W], BF16,
                              kind="ExternalOutput")
    ccin = nc.dram_tensor("ccin", [C, 9], F32)
    ccout = nc.dram_tensor("ccout", [C, 9], F32)
    padrows = nc.dram_tensor("padrows", [2 * CH, W], F32R)

    with tile.TileContext(nc) as tc:
        with tc.tile_pool(name="persist", bufs=1) as pp:
            # ---------- constants to SBUF ----------
            def ctiles(name, cols, tiling):
                ts = []
                for (r0, rn) in tiling:
                    t = pp.tile([rn, cols], F32R, tag=f"{name}_{r0}", name=f"{name}_{r0}")
                    nc.sync.dma_start(t[:, :],
                                      din[name][r0:r0 + rn, :].bitcast(F32R))
                    ts.append(t)
                return ts

            FHt = ctiles("FH", 300, PT)
            FCt = ctiles("FC", HP, PT)
            FSt = ctiles("FS", HP, PT)
            FSnt = ctiles("FSneg", HP, PT)
            GHCt = ctiles("GHC", HP, UT)
            GHSt = ctiles("GHS", HP, UT)
            GHSnt = ctiles("GHSneg", HP, UT)
            C2t = ctiles("C2", HP, PT)
            S2nt = ctiles("S2neg", HP, PT)

            def cload(name, shape, dt=F32R):
                t = pp.tile(list(shape), dt, tag=name, name=name)
                src = din[name][:]
                if src.tensor.dtype != dt:
                    src = src.bitcast(dt)
                nc.sync.dma_start(t[:], src)
                return t

            E3r = cload("E3r", (3, HP))
            E3i = cload("E3i", (3, HP))
            E3ip = cload("E3ip", (3, HP))
            Eu3c = cload("Eu3c", (3, NU))
            Eu3s = cload("Eu3s", (3, NU))
            Eu3sn = cload("Eu3sneg", (3, NU))
            E21r = cload("E21r", (KS, HP), F32)
            E21i = cload("E21i", (KS, HP), F32)
            Eu21c = cload("Eu21c", (KS, NU), F32)
            Eu21s = cload("Eu21s", (KS, NU), F32)
            Eu21sn = cload("Eu21sneg", (KS, NU), F32)
            kerTs = cload("kerT", (KS, KS), F32)
            wred_s = cload("wred24", (NF, 24))
            wg_s = {k: cload(k, (96, 32)) for k in ("wg1", "wg2", "wg3")}
            wg4_s = cload("wg4p", (C, CH))
            wexp_s = cload("wexp8", (128, NF))
            bred_s = cload("bred24", (24, 1), F32)
            bg_s = {k: cload(k, (C, 1), F32) for k in ("bg1", "bg2", "bg3")}
            bg4_s = cload("bg4p", (CH, 1), F32)
            bexp_s = cload("bexp", (NF, 1), F32)
            selm_s = cload("selm", (C, 2), F32)
            wrbx_s = cload("wrbx", (C, 3, 375), F32)

            # ---------- persistent activations ----------
            cls_g = pp.tile([128, 17 * W], F32R, tag="cls_g")
            clsF = pp.tile([128, 16 * W], F32R, tag="clsF")
            g1p = pp.tile([128, 17 * W], F32R, tag="g1p")
            g2p = pp.tile([128, 17 * W], F32R, tag="g2p")
            clear = pp.tile([128, 16 * W], F32R, tag="clear")
            Krt = [pp.tile([rn, HP], F32, tag=f"Kr{r0}", name=f"Kr{r0}") for r0, rn in UT]
            Kit = [pp.tile([rn, HP], F32, tag=f"Ki{r0}", name=f"Ki{r0}") for r0, rn in UT]
            KD2t = [pp.tile([rn, HP], F32, tag=f"KD2{r0}", name=f"KD2{r0}") for r0, rn in UT]
            kp8 = pp.tile([CH, 9], F32R, tag="kp8")
            Tt = pp.tile([C, 125, 3], F32, tag="Tt")

            # ============ stage A: reduce conv ============
            with tc.tile_pool(name="sA", bufs=3) as pa, \
                 tc.tile_pool(name="psA", bufs=4, space="PSUM") as ppa:
                for kb in range(16):
                    xs = pa.tile([NF, 16 * W], F32R, tag="xs")
                    nc.sync.dma_start(xs[:, :],
                                      x64[:, 16 * kb:16 * kb + 16, :])
                    for ch in range(4):
                        ps = ppa.tile([24, 4 * W], F32, tag="psred")
                        for hf in range(2):
                            nc.tensor.matmul(
                                ps[:, hf * 512:(hf + 1) * 512], wred_s[:, :],
                                xs[:, ch * 1024 + hf * 512:
                                   ch * 1024 + (hf + 1) * 512],
                                start=True, stop=True)
                        bt = pa.tile([24, 4 * W], F32R, tag="bounce")
                        if ch % 2 == 0:
                            nc.scalar.activation(bt[:, :], ps[:, :],
                                                 AF.Identity,
                                                 bias=bred_s[:, 0:1])
                        else:
                            nc.vector.tensor_scalar_add(bt[:, :], ps[:, :],
                                                        bred_s[:, 0:1])
                        y0 = 16 * kb + 4 * ch
                        nc.sync.dma_start(
                            clsF[8 * kb:8 * kb + 8,
                                 4 * ch * W:(4 * ch + 4) * W],
                            bt[16:24, :])
                        for r in range(4):
                            y = y0 + r
                            if y >= 136:
                                continue
                            blk, off = divmod(y, 17)
                            nc.sync.dma_start(
                                cls_g[16 * blk:16 * blk + 16,
                                      off * W:(off + 1) * W],
                                bt[0:16, r * W:(r + 1) * W])

            # ============ Kf (per sample) ============
            with tc.tile_pool(name="skf", bufs=1) as pk, \
                 tc.tile_pool(name="pskf", bufs=2, space="PSUM") as ppk:
                psG = ppk.tile([KS, 1024], F32, tag="psG21")
                nc.tensor.matmul(psG[:, 0:HP], kerTs[:, :], E21r[:, :],
                                 start=True, stop=True)
                nc.tensor.matmul(psG[:, 512:512 + HP], kerTs[:, :],
                                 E21i[:, :], start=True, stop=True)
                G21 = pk.tile([KS, 2 * HP], F32, tag="G21")
                nc.vector.tensor_copy(G21[:, 0:HP], psG[:, 0:HP])
                nc.vector.tensor_copy(G21[:, HP:2 * HP],
                                      psG[:, 512:512 + HP])
                for it, (u0, un) in enumerate(UT):
                    psr = ppk.tile([un, HP], F32, tag="psKr")
                    psi = ppk.tile([un, HP], F32, tag="psKi")
                    nc.tensor.matmul(psr[:, :], Eu21c[:, u0:u0 + un],
                                     G21[:, 0:HP], start=True, stop=False)
                    nc.tensor.matmul(psr[:, :], Eu21sn[:, u0:u0 + un],
                                     G21[:, HP:2 * HP], start=False,
                                     stop=True)
                    nc.tensor.matmul(psi[:, :], Eu21c[:, u0:u0 + un],
                                     G21[:, HP:2 * HP], start=True,
                                     stop=False)
                    nc.tensor.matmul(psi[:, :], Eu21s[:, u0:u0 + un],
                                     G21[:, 0:HP], start=False, stop=True)
                    nc.vector.tensor_copy(Krt[it][:, :], psr[:, :])
                    nc.vector.tensor_copy(Kit[it][:, :], psi[:, :])
                    t1 = pk.tile([128, HP], F32, tag="kd_t1")
                    nc.scalar.activation(t1[0:un, :], psr[:, :], AF.Square)
                    nc.scalar.activation(KD2t[it][:, :], psi[:, :], AF.Square)
                    nc.vector.tensor_add(KD2t[it][:, :], KD2t[it][:, :],
                                         t1[0:un, :])

            # ============ g-chain (local frame, split across pair) ========
            def conv_layer(li, src, w_s, b_s, in_cols, out_rows, dst):
                out_cols = in_cols - 2
                with tc.tile_pool(name=f"g{li}", bufs=3) as pg, \
                     tc.tile_pool(name=f"psg{li}", bufs=4,
                                  space="PSUM") as ppg:
                    nblk = (out_rows + 16) // 17
                    for bk in range(nblk):
                        r0 = 17 * bk
                        rows = min(17, out_rows - r0)
                        r96 = pg.tile([96, 19 * W], F32R, tag=f"r96_{li}")
                        for dyy in range(2):
                            need = rows + 2 if dyy == 0 else rows
                            got = 0
                            while got < need:
                                y = r0 + dyy + got
                                sb, so = divmod(y, 17)
                                n = min(17 - so, need - got)
                                for dx in range(3):
                                    nc.sync.dma_start(
                                        r96[48 * dyy + 16 * dx:
                                            48 * dyy + 16 * dx + 16,
                                            got * W:(got + n) * W - dx],
                                        src[16 * sb:16 * sb + 16,
                                            so * W + dx:(so + n) * W])
                                got += n
                        for c0 in range(0, rows, 2):
                            rr = min(2, rows - c0)
                            ps = ppg.tile([16, 2, out_cols], F32,
                                          tag=f"ps_{li}")
                            rhs3 = r96[:, c0 * W:(c0 + rr) * W].rearrange(
                                "p (r x) -> p r x", r=rr)
                            nc.tensor.matmul(
                                ps[:, 0:rr, :], w_s[:, 0:16],
                                rhs3[:, :, 0:out_cols],
                                start=True, stop=False)
                            rhs2 = r96[0:48, (c0 + 2) * W:
                                       (c0 + 2 + rr) * W].rearrange(
                                "p (r x) -> p r x", r=rr)
                            nc.tensor.matmul(
                                ps[:, 0:rr, :], w_s[0:48, 16:32],
                                rhs2[:, :, 0:out_cols],
                                start=False, stop=True)
                            if dst is not None:
                                bt = pg.tile([16, 2, out_cols], F32R,
                                             tag=f"bt_{li}")
                                nc.scalar.activation(
                                    bt[:, 0:rr, :], ps[:, 0:rr, :],
                                    AF.Lrelu, bias=b_s[:, 0:1], alpha=0.1)
                                for r in range(rr):
                                    yo = r0 + c0 + r
                                    db_, do_ = divmod(yo, 17)
                                    nc.sync.dma_start(
                                        dst[16 * db_:16 * db_ + 16,
                                            do_ * W:do_ * W + out_cols],
                                        bt[:, r, :])
                            else:
                                # g3: overlapping column-bin sums from PSUM
                                for r in range(rr):
                                    yo = r0 + c0 + r
                                    full = ps[:, :, :]
                                    binv = _apv(full, r * out_cols,
                                                [list(full.ap[0]),
                                                 [83, 3], [1, 84]])
                                    nc.vector.tensor_reduce(
                                        Tt[:, yo, :], binv,
                                        mybir.AxisListType.X, ALU.add)

            conv_layer(1, cls_g, wg_s["wg1"], bg_s["bg1"], 256, 129, g1p)
            conv_layer(2, g1p, wg_s["wg2"], bg_s["bg2"], 254, 127, g2p)
            conv_layer(3, g2p, wg_s["wg3"], bg_s["bg3"], 252, 125, None)

            # ---- pool partials, AllReduce, kernel_P ----
            with tc.tile_pool(name="spool", bufs=1) as pq, \
                 tc.tile_pool(name="pspool", bufs=2, space="PSUM") as ppq:
                Sp = pq.tile([C, 3, 3], F32, tag="Spart")
                tmp = pq.tile([C, 125, 3], F32, tag="ptmp")
                for ib in range(3):
                    nc.vector.tensor_tensor(
                        tmp[:, :, :], Tt[:, :, :],
                        wrbx_s[:, ib, :].rearrange("p (y j) -> p y j", j=3),
                        ALU.mult)
                    tv = tmp[:, :, :]
                    swapped = _apv(tv, 0, [list(tv.ap[0]), [1, 3], [3, 125]])
                    nc.vector.tensor_reduce(Sp[:, ib, :], swapped,
                                            mybir.AxisListType.X, ALU.add)
                nc.sync.dma_start(ccin[:, :], Sp[:, :, :])
                nc.gpsimd.collective_compute(
                    "AllReduce", ALU.add,
                    replica_groups=[[0, 1], [2, 3], [4, 5], [6, 7]],
                    ins=[ccin[:, :]], outs=[ccout[:, :]])
                pooled = pq.tile([C, 9], F32, tag="pooled")
                nc.sync.dma_start(pooled[:, :], ccout[:, :])
                pflip = pq.tile([C, 9], F32, tag="pflip")
                for ib in range(3):
                    nc.vector.tensor_copy(
                        pflip[:, 3 * ib:3 * ib + 3],
                        pooled[:, 3 * (2 - ib):3 * (2 - ib) + 3])
                pmine = pq.tile([C, 16], F32R, tag="pmine")
                nc.vector.tensor_scalar_mul(pmine[:, 9:16], pooled[:, 0:7],
                                            0.0)
                psel = pq.tile([C, 9], F32, tag="psel")
                nc.vector.tensor_scalar_mul(psel[:, :], pooled[:, :],
                                            selm_s[:, 0:1])
                nc.vector.scalar_tensor_tensor(
                    psel[:, :], pflip[:, :], selm_s[:, 1:2], psel[:, :],
                    ALU.mult, ALU.add)
                # add b_g3 (pool commutes with the bias)
                nc.vector.tensor_scalar_add(pmine[:, 0:9], psel[:, :],
                                            bg_s["bg3"][:, 0:1])
                psk = ppq.tile([CH, 16], F32, tag="psk")
                nc.tensor.matmul(psk[:, :], wg4_s[:, :], pmine[:, :],
                                 start=True, stop=True)
                kpe = pq.tile([CH, 9], F32, tag="kpe")
                nc.scalar.activation(kpe[:, :], psk[:, 0:9], AF.Exp,
                                     bias=bg4_s[:, 0:1])
                nsum = pq.tile([CH, 1], F32, tag="nsum")
                nc.vector.tensor_reduce(nsum[:, :], kpe[:, :],
                                        mybir.AxisListType.X, ALU.add,
                                        negate=True)
                nmean = pq.tile([CH, 1], F32, tag="nmean")
                nc.scalar.mul(nmean[:, :], nsum[:, :], 1.0 / 9.0)
                nc.vector.tensor_scalar_add(kp8[:, :], kpe[:, :],
                                            nmean[:, 0:1])

            # ============ FFT / Wiener per channel ============
            with tc.tile_pool(name="fft", bufs=2) as pf, \
                 tc.tile_pool(name="fftx", bufs=3) as pfx, \
                 tc.tile_pool(name="psf", bufs=2, space="PSUM") as ppf, \
                 tc.tile_pool(name="psf1", bufs=2, space="PSUM") as ppf1:
                for cix in range(CH):
                    # ---- build padded X ----
                    Xt = [pfx.tile([rn, HP], F32R, tag=f"X{r0}", name=f"X{r0}")
                          for r0, rn in PT]
                    for sb in range(16):
                        srow = clsF[8 * sb + cix:8 * sb + cix + 1, :]
                        sv = srow.rearrange("p (y x) -> p y x", x=W)
                        yd0 = 21 + 16 * sb
                        done = 0
                        while done < 16:
                            yd = yd0 + done
                            ti = 0 if yd < 128 else (1 if yd < 256 else 2)
                            t0 = PT[ti][0]
                            n = min(16 - done, t0 + PT[ti][1] - yd)
                            nc.sync.dma_start(
                                Xt[ti][yd - t0:yd - t0 + n, 21:21 + W],
                                sv[0:1, done:done + n, :])
                            done += n
                    nc.sync.dma_start(padrows[2 * cix:2 * cix + 1, :],
                                      clsF[cix:cix + 1, 0:W])
                    nc.sync.dma_start(
                        padrows[2 * cix + 1:2 * cix + 2, :],
                        clsF[8 * 15 + cix:8 * 15 + cix + 1, 15 * W:16 * W])
                    nc.sync.dma_start(
                        Xt[0][0:21, 21:21 + W],
                        padrows[2 * cix:2 * cix + 1, :]
                        .broadcast_to([21, W]))
                    nc.sync.dma_start(
                        Xt[2][21:42, 21:21 + W],
                        padrows[2 * cix + 1:2 * cix + 2, :]
                        .broadcast_to([21, W]))
                    for ti, (r0, rn) in enumerate(PT):
                        # col pads: out = in*0 + colvalue  (per-partition
                        # scalar broadcast along free dim)
                        nc.vector.tensor_scalar(
                            Xt[ti][:, 0:21], Xt[ti][:, 21:42], 0.0,
                            Xt[ti][:, 21:22].bitcast(F32), ALU.mult,
                            ALU.add)
                        nc.vector.tensor_scalar(
                            Xt[ti][:, 277:HP], Xt[ti][:, 255:276], 0.0,
                            Xt[ti][:, 276:277].bitcast(F32), ALU.mult,
                            ALU.add)

                    # ---- stage 1: A^T[j, u] ----
                    At = [pfx.tile([rn, 300], F32R, tag=f"At{r0}", name=f"At{r0}")
                          for r0, rn in PT]
                    for jt, (j0, jn) in enumerate(PT):
                        psA = ppf.tile([128, 300], F32, tag="psPr", name="psA")[0:jn]
                        for it in range(3):
                            nc.tensor.matmul(psA[:, :],
                                             Xt[it][:, j0:j0 + jn],
                                             FHt[it][:, :],
                                             start=(it == 0), stop=(it == 2))
                        nc.scalar.copy(At[jt][:, :], psA[:, :])

                    # ---- Pf (contract r first; P3 in natural layout) ----
                    P3 = pf.tile([3, 3], F32R, tag="P3")
                    nc.sync.dma_start(
                        P3[:, :],
                        kp8[cix:cix + 1, :].rearrange("p (r s) -> p r s",
                                                      s=3))
                    psGur = ppf1.tile([128, HP], F32, tag="psBr",
                                      name="psGur")[0:3]
                    psGui = ppf1.tile([128, HP], F32, tag="psBi",
                                      name="psGui")[0:3]
                    nc.tensor.matmul(psGur[:, 0:NU], P3[:, :], Eu3c[:, :],
                                     start=True, stop=True)
                    nc.tensor.matmul(psGui[:, 0:NU], P3[:, :], Eu3s[:, :],
                                     start=True, stop=True)
                    G3 = pf.tile([3, 2 * NU], F32R, tag="G3")
                    nc.vector.tensor_copy(G3[:, 0:NU], psGur[:, 0:NU])
                    nc.vector.tensor_copy(G3[:, NU:2 * NU], psGui[:, 0:NU])

                    # ---- stage 2 + Wiener per u-tile ----
                    Zr = [pf.tile([rn, HP], F32R, tag=f"Zr{r0}", name=f"Zr{r0}")
                          for r0, rn in UT]
                    Zi = [pf.tile([rn, HP], F32R, tag=f"Zi{r0}", name=f"Zi{r0}")
                          for r0, rn in UT]
                    for it, (u0, un) in enumerate(UT):
                        psPr = ppf.tile([128, HP], F32, tag="psPr", name="psPr")[0:un]
                        psPi = ppf.tile([128, HP], F32, tag="psPi", name="psPi")[0:un]
                        nc.tensor.matmul(psPr[:, :],
                                         G3[:, u0:u0 + un],
                                         E3r[:, :], start=True, stop=False)
                        nc.tensor.matmul(psPr[:, :],
                                         G3[:, NU + u0:NU + u0 + un],
                                         E3ip[:, :], start=False, stop=True)
                        nc.tensor.matmul(psPi[:, :],
                                         G3[:, u0:u0 + un],
                                         E3i[:, :], start=True, stop=False)
                        nc.tensor.matmul(psPi[:, :],
                                         G3[:, NU + u0:NU + u0 + un],
                                         E3r[:, :], start=False, stop=True)
                        psBr = ppf1.tile([128, HP], F32, tag="psBr", name="psBr")[0:un]
                        psBi = ppf1.tile([128, HP], F32, tag="psBi", name="psBi")[0:un]
                        for jt, (j0, jn) in enumerate(PT):
                            Ar = At[jt][:, u0:u0 + un]
                            Ai = At[jt][:, 150 + u0:150 + u0 + un]
                            nc.tensor.matmul(psBr[:, :], Ar, FCt[jt][:, :],
                                             start=(jt == 0), stop=False)
                            nc.tensor.matmul(psBr[:, :], Ai, FSt[jt][:, :],
                                             start=False, stop=(jt == 2))
                            nc.tensor.matmul(psBi[:, :], Ai, FCt[jt][:, :],
                                             start=(jt == 0), stop=False)
                            nc.tensor.matmul(psBi[:, :], Ar, FSnt[jt][:, :],
                                             start=False, stop=(jt == 2))
                        sq1 = pf.tile([128, HP], F32, tag="sq1")
                        sq2 = pf.tile([128, HP], F32, tag="sq2")
                        nc.scalar.activation(sq1[0:un, :], psPr[:, :],
                                             AF.Square)
                        nc.scalar.activation(sq2[0:un, :], psPi[:, :],
                                             AF.Square)
                        nc.vector.tensor_add(sq1[0:un, :], sq1[0:un, :],
                                             sq2[0:un, :])
                        nc.vector.tensor_add(sq1[0:un, :], sq1[0:un, :],
                                             KD2t[it][:, :])
                        rec = pf.tile([128, HP], F32, tag="rec")
                        nc.vector.reciprocal(rec[0:un, :], sq1[0:un, :])
                        m1 = pf.tile([128, HP], F32, tag="m1")
                        m2 = pf.tile([128, HP], F32, tag="m2")
                        nc.vector.tensor_tensor(m1[0:un, :], psBr[:, :],
                                                Krt[it][:, :], ALU.mult)
                        nc.vector.tensor_tensor(m2[0:un, :], psBi[:, :],
                                                Kit[it][:, :], ALU.mult)
                        nc.vector.tensor_add(m1[0:un, :], m1[0:un, :],
                                             m2[0:un, :])
                        nc.vector.tensor_tensor(Zr[it][:, :], m1[0:un, :],
                                                rec[0:un, :], ALU.mult)
                        nc.vector.tensor_tensor(m1[0:un, :], psBi[:, :],
                                                Krt[it][:, :], ALU.mult)
                        nc.vector.tensor_tensor(m2[0:un, :], psBr[:, :],
                                                Kit[it][:, :], ALU.mult)
                        nc.vector.tensor_tensor(m1[0:un, :], m1[0:un, :],
                                                m2[0:un, :], ALU.subtract)
                        nc.vector.tensor_tensor(Zi[it][:, :], m1[0:un, :],
                                                rec[0:un, :], ALU.mult)

                    # ---- inverse stage 1: V^T[v, n] ----
                    Vr = [pf.tile([rn, HP], F32R, tag=f"Vr{r0}", name=f"Vr{r0}")
                          for r0, rn in PT]
                    Vi = [pf.tile([rn, HP], F32R, tag=f"Vi{r0}", name=f"Vi{r0}")
                          for r0, rn in PT]
                    for vt, (v0, vn) in enumerate(PT):
                        psVr = ppf.tile([128, HP], F32, tag="psPr", name="psVr")[0:vn]
                        psVi = ppf.tile([128, HP], F32, tag="psPi", name="psVi")[0:vn]
                        for it, (u0, un) in enumerate(UT):
                            zr = Zr[it][:, v0:v0 + vn]
                            zi = Zi[it][:, v0:v0 + vn]
                            nc.tensor.matmul(psVr[:, :], zr, GHCt[it][:, :],
                                             start=(it == 0), stop=False)
                            nc.tensor.matmul(psVr[:, :], zi, GHSnt[it][:, :],
                                             start=False, stop=(it == 1))
                            nc.tensor.matmul(psVi[:, :], zi, GHCt[it][:, :],
                                             start=(it == 0), stop=False)
                            nc.tensor.matmul(psVi[:, :], zr, GHSt[it][:, :],
                                             start=False, stop=(it == 1))
                        nc.scalar.copy(Vr[vt][:, :], psVr[:, :])
                        nc.vector.tensor_copy(Vi[vt][:, :], psVi[:, :])

                    # ---- inverse stage 2 + crop + remap ----
                    for nt in range(2):
                        n0 = 21 + 128 * nt
                        psD = ppf.tile([128, HP], F32, tag="psPr", name="psD")
                        for vt, (v0, vn) in enumerate(PT):
                            nc.tensor.matmul(psD[:, :],
                                             Vr[vt][:, n0:n0 + 128],
                                             C2t[vt][:, :],
                                             start=(vt == 0), stop=False)
                            nc.tensor.matmul(psD[:, :],
                                             Vi[vt][:, n0:n0 + 128],
                                             S2nt[vt][:, :],
                                             start=False, stop=(vt == 2))
                        deb = pf.tile([128, W], F32R, tag="deb")
                        nc.vector.tensor_copy(deb[:, :], psD[:, 21:277])
                        dv = clear[:, :]
                        dst = _apv(dv, (cix + 64 * nt) * (16 * W),
                                   [[8 * 16 * W, 8], [W, 16], [1, W]])
                        nc.sync.dma_start(dst, deb[:, :])

            # ============ expand (rank-8 partial) ============
            with tc.tile_pool(name="sE", bufs=4) as pe, \
                 tc.tile_pool(name="psE", bufs=4, space="PSUM") as ppe:
                for yb in range(16):
                    stg = pe.tile([CH, 16 * W], F32R, tag="stg")
                    nc.sync.dma_start(stg[:, :],
                                      clear[8 * yb:8 * yb + 8, :])
                    for ck in range(8):
                        psE = ppe.tile([NF, 512], F32, tag="psE")
                        nc.tensor.matmul(
                            psE[:, :], wexp_s[0:8, :],
                            stg[:, 512 * ck:512 * (ck + 1)],
                            start=True, stop=True)
                        ob = pe.tile([NF, 512], BF16, tag="ob")
                        if ck % 2 == 0:
                            nc.vector.tensor_scalar_add(ob[:, :], psE[:, :],
                                                        bexp_s[:, 0:1])
                        else:
                            nc.scalar.activation(ob[:, :], psE[:, :],
                                                 AF.Identity,
                                                 bias=bexp_s[:, 0:1])
                        y = 16 * yb + 2 * ck
                        nc.sync.dma_start(out_part[:, y:y + 2, :],
                                          ob[:, :])

    return nc


# ---------------------------------------------------------------- host
def _core_inputs(inputs, cid):
    b, h = cid // 2, cid % 2
    flip = (h == 1)
    x = inputs["x"][b]
    ker = inputs["kernel"][b, 0]
    if flip:
        x = np.flip(x, axis=1)
        ker = np.flip(ker, axis=0)
    wr = inputs["w_reduce"][:, :, 0, 0]          # [16, 64]
    wred24 = np.zeros((NF, 24), np.float32)
    wred24[:, 0:16] = wr.T
    wred24[:, 16:24] = wr[8 * h:8 * h + 8].T

    def packg(wg):                               # [o,c,3,3] -> [96, 32]
        w = np.flip(wg, axis=2) if flip else wg
        out = np.zeros((96, 32), np.float32)
        for dy in range(3):
            for dx in range(3):
                for cc in range(16):
                    if dy < 2:
                        out[48 * dy + 16 * dx + cc, 0:16] = w[:, cc, dy, dx]
                    else:
                        out[16 * dx + cc, 16:32] = w[:, cc, dy, dx]
        return out

    wexp = inputs["w_expand"][:, :, 0, 0]        # [64, 16]
    wexp8 = np.tile(wexp[:, 8 * h:8 * h + 8].T, (16, 1)).astype(np.float32)

    m = {
        "x64": np.ascontiguousarray(x, np.float32),
        "kerT": np.ascontiguousarray(ker.T, np.float32),
        "wred24": wred24,
        "wg1": packg(inputs["w_g1"]),
        "wg2": packg(inputs["w_g2"]),
        "wg3": packg(inputs["w_g3"]),
        "wg4p": np.ascontiguousarray(
            inputs["w_g4"][8 * h:8 * h + 8, :, 0, 0].T, np.float32),
        "wexp8": wexp8,
        "bred24": np.concatenate(
            [inputs["b_reduce"], inputs["b_reduce"][8 * h:8 * h + 8]]
        ).reshape(24, 1).astype(np.float32),
        "bg1": inputs["b_g1"].reshape(C, 1).astype(np.float32),
        "bg2": inputs["b_g2"].reshape(C, 1).astype(np.float32),
        "bg3": inputs["b_g3"].reshape(C, 1).astype(np.float32),
        "bg4p": inputs["b_g4"][8 * h:8 * h + 8].reshape(CH, 1)
                .astype(np.float32),
        "bexp": (inputs["b_expand"] if h == 0
                 else np.zeros(NF)).reshape(NF, 1).astype(np.float32),
        "selm": np.tile(np.array([[1.0, 0.0]] if h == 0 else [[0.0, 1.0]],
                                 np.float32), (C, 1)),
        "wrbx": _row_weights(h),
    }
    return m


# ---------------------------------------------------------------- runtime
def _make_exec(builder=None):
    """Build + compile the SPMD executable (one bass_exec per module)."""
    import concourse.mybir as mybir
    import jax
    from jax.sharding import Mesh, PartitionSpec
    from jax.experimental.shard_map import shard_map
    from concourse.bass2jax import (_bass_exec_p, partition_id_tensor,
                                    install_neuronx_cc_hook)

    install_neuronx_cc_hook()
    nc = (builder or _build_program)()

    partition_name = (nc.partition_id_tensor.name
                      if nc.partition_id_tensor else None)
    in_names, out_names, out_avals = [], [], []
    for alloc in nc.m.functions[0].allocations:
        if not isinstance(alloc, mybir.MemoryLocationSet):
            continue
        if alloc.kind == "ExternalInput":
            name = alloc.memorylocations[0].name
            if name != partition_name:
                in_names.append(name)
        elif alloc.kind == "ExternalOutput":
            out_names.append(alloc.memorylocations[0].name)
            out_avals.append(jax.core.ShapedArray(
                tuple(alloc.tensor_shape), mybir.dt.np(alloc.dtype)))
    bind_names = tuple(in_names + ([partition_name] if partition_name else []))

    def _body(*args):
        ops = list(args) + ([partition_id_tensor()] if partition_name else [])
        return tuple(_bass_exec_p.bind(
            *ops, out_avals=tuple(out_avals), in_names=bind_names,
            out_names=tuple(out_names),
            lowering_input_output_aliases=(),
            sim_require_finite=False, sim_require_nnan=False, nc=nc))

    devices = jax.devices()[:N_CORES]
    mesh = Mesh(np.asarray(devices), ("core",))
    sharded = jax.jit(shard_map(
        _body, mesh=mesh,
        in_specs=(PartitionSpec("core"),) * len(in_names),
        out_specs=(PartitionSpec("core"),) * len(out_names),
        check_rep=False))

    shapes = {alloc.memorylocations[0].name:
              (tuple(alloc.tensor_shape), mybir.dt.np(alloc.dtype))
              for alloc in nc.m.functions[0].allocations
              if isinstance(alloc, mybir.MemoryLocationSet)
              and alloc.kind == "ExternalInput"}
    avals = [jax.ShapeDtypeStruct((N_CORES * shapes[n][0][0],)
                                  + shapes[n][0][1:], shapes[n][1])
             for n in in_names]
    compiled = sharded.lower(*avals).compile()
    return {"compiled": compiled, "in_names": in_names,
            "out_avals": out_avals, "mesh": mesh}


def _get_rt():
    if "exec" not in _RT:
        _RT["exec"] = _make_exec()
    return _RT["exec"]


def _concat_inputs(inputs, rt):
    in_maps = [_core_inputs(inputs, cid) for cid in range(N_CORES)]
    return [np.concatenate([np.asarray(in_maps[c][n])
                            for c in range(N_CORES)], axis=0)
            for n in rt["in_names"]]


def stage_inputs(inputs, rt=None):
    """Transfer concatenated inputs to device, returning device arrays."""
    import jax
    from jax.sharding import PartitionSpec, NamedSharding
    rt = rt or _get_rt()
    cat = _concat_inputs(inputs, rt)
    if "stager" not in _RT:
        sh = tuple(NamedSharding(rt["mesh"], PartitionSpec("core"))
                   for _ in cat)
        _RT["stager"] = jax.jit(lambda *a: a, out_shardings=sh)
    staged = _RT["stager"](*cat)
    jax.block_until_ready(staged)
    return staged


def _assemble(out_np):
    out = np.zeros((B, NF, H, W), np.float32)
    arr = np.asarray(out_np, np.float32).reshape(N_CORES, NF, H, W)
    for b in range(B):
        out[b] = arr[2 * b] + np.flip(arr[2 * b + 1], axis=1)
    return out


def kernel(**inputs):
    inputs = {k: np.asarray(v) for k, v in inputs.items()}
    rt = _get_rt()
    cat = _concat_inputs(inputs, rt)
    outs = rt["compiled"](*cat)
    return _assemble(outs[0])


def run_staged(staged, k=1, rt=None):
    """Async-dispatch the executable k times back-to-back on pre-staged
    device inputs; the per-core execution queue serializes them, so
    (T(k) - T(1)) / (k - 1) is the per-execution NEFF time."""
    import jax
    rt = rt or _get_rt()
    outs = None
    for _ in range(k):
        outs = rt["compiled"](*staged)
    jax.block_until_ready(outs)
    return outs
